# revision 10
# baseline (speedup 1.0000x reference)
"""Distributed GQA attention kernel for Trainium2 (8 NeuronCores).

Sharding: 2-way data parallel over batch x 4-way tensor parallel over heads.
Core c handles batch b = c // 4 and head group g = c % 4 (8 q-heads, 2 kv-heads).
Each core computes a full-size partial of the output (its head group pushed
through Wo); the host sums the 4 partials per batch. No on-device collective.

Device-side layout is feature-major (Q^T/K^T: [feature partitions, T free]) so
projections consume the host-pre-transposed x^T directly, attention scores are
computed transposed (S^T[tk, tq]) so softmax(P)@V needs no transposes, and the
softmax denominator comes free from an appended ones-column on V.

Schedule: x streams in 512-token column windows; each phase nt runs
K/V-projection prefetch for window nt+1, Q-projection + attention for q-tile
nt, and interleaves the previous tile's Wo matmuls into the per-block PE gaps
of the ACT(exp)-paced attention stream. V is projected feature-major (big
moving dim) and flipped token-major by DMA-engine transposes.
"""

import numpy as np
import ml_dtypes
from collections import deque
from contextlib import ExitStack

import concourse.bass as bass
from concourse import bacc
import concourse.mybir as mybir
import concourse.tile as tile
from concourse.bass_utils import run_bass_kernel_spmd

BF16 = mybir.dt.bfloat16
F32 = mybir.dt.float32
AF = mybir.ActivationFunctionType

P = 128
B, T, D = 2, 2048, 2048
NUM_HEADS, NUM_KV_HEADS, HD = 32, 8, 64
FQ = 512          # q features per core (8 heads x 64)
DKV = 128         # kv features per core (2 kv heads x 64)
KO = D // P       # 16 contraction tiles over d_model
NT = T // 512     # 4 tiles of 512 along T
SCALE = 1.0 / np.sqrt(HD)
ROPE_BASE = 10000.0
# local head order inside the 512 q-features: pairs (j, j+4) so that the two
# heads in partition tile j sit at bases 0/64 matching kv heads 0/1 in K^T
PERM_Q = [0, 4, 1, 5, 2, 6, 3, 7]

_nc_cache = {}


def build_nc():
    if "nc" in _nc_cache:
        return _nc_cache["nc"]
    nc = bacc.Bacc()
    xS = nc.declare_dram_parameter("xS", [P, NT, KO, 512], BF16, isOutput=False)
    wqS = nc.declare_dram_parameter("wqS", [P, 4, KO, P], BF16, isOutput=False)
    wkS = nc.declare_dram_parameter("wkS", [P, KO, DKV], BF16, isOutput=False)
    wvS = nc.declare_dram_parameter("wvS", [P, KO, DKV], BF16, isOutput=False)
    woS = nc.declare_dram_parameter("woS", [P, 4, D], BF16, isOutput=False)
    cosd = nc.declare_dram_parameter("cosT", [P, T], BF16, isOutput=False)
    sind = nc.declare_dram_parameter("sinT", [P, T], BF16, isOutput=False)
    mskd = nc.declare_dram_parameter("tri", [P, P], BF16, isOutput=False)
    y = nc.declare_dram_parameter("y", [T, D], BF16, isOutput=True)

    with tile.TileContext(nc) as tc:
        with ExitStack() as ctx:
            const = ctx.enter_context(tc.tile_pool(name="const", bufs=1))
            work = ctx.enter_context(tc.tile_pool(name="work", bufs=4))
            otp = ctx.enter_context(tc.tile_pool(name="otp", bufs=2))
            pexp = ctx.enter_context(tc.tile_pool(name="pexp", bufs=8))
            rrp = ctx.enter_context(tc.tile_pool(name="rrp", bufs=2))
            dramp = ctx.enter_context(tc.tile_pool(name="dramp", bufs=2, space="DRAM"))
            big_ps = ctx.enter_context(tc.tile_pool(name="bigps", bufs=2, space="PSUM"))
            pv_ps = ctx.enter_context(tc.tile_pool(name="pvps", bufs=1, space="PSUM"))
            s_ps = ctx.enter_context(tc.tile_pool(name="sps", bufs=2, space="PSUM"))

            # ---- persistent tiles ----
            wk_sb = const.tile([P, KO, DKV], BF16, tag="wk")
            wv_sb = const.tile([P, KO, DKV], BF16, tag="wv")
            wq_sb = const.tile([P, KO, FQ], BF16, tag="wq")
            wo_sb = const.tile([P, 4, D], BF16, tag="wo")
            cos_sb = const.tile([P, T], BF16, tag="cos")
            sin_sb = const.tile([P, T], BF16, tag="sin")
            tri_sb = const.tile([P, P], BF16, tag="tri")
            x_sb = const.tile([P, KO, T], BF16, tag="x")
            kt = const.tile([P, T], BF16, tag="kt")
            v_sb = const.tile([P, 16, 130], BF16, tag="v")
            ones_sb = const.tile([1, 64], BF16, tag="ones")
            qts = {j: const.tile([P, T], BF16, tag=f"qt{j}", name=f"qt{j}")
                   for j in range(4)}

            nc.gpsimd.memset(ones_sb[:], 1.0)
            nc.gpsimd.memset(v_sb[:, :, 64:65], 1.0)
            nc.gpsimd.memset(v_sb[:, :, 129:130], 1.0)

            # ---- DMA emission order = startup priority ----
            # Bulk loads go on the Scalar engine's HWDGE ring so they never
            # queue ahead of the latency-critical small DMAs (rope rotates,
            # softmax-denominator bounces) on the Sync ring. wk + first x
            # window unblock K proj; the rest streams behind compute.
            nc.scalar.dma_start(wk_sb[:], wkS[:, :, :])
            for q4 in range(4):
                nc.scalar.dma_start(x_sb[:, q4 * 4:(q4 + 1) * 4, 0:512],
                                    xS[:, 0, q4 * 4:(q4 + 1) * 4, :])
            nc.scalar.dma_start(cos_sb[:], cosd[:])
            nc.scalar.dma_start(sin_sb[:], sind[:])
            nc.scalar.dma_start(wv_sb[:], wvS[:, :, :])
            nc.scalar.dma_start(wq_sb[:, :, 0:P], wqS[:, 0])
            nc.scalar.dma_start(tri_sb[:], mskd[:])
            for h in range(2):
                nc.scalar.dma_start(x_sb[:, h * 8:(h + 1) * 8, 512:1024],
                                    xS[:, 1, h * 8:(h + 1) * 8, :])
            for j in (1, 2, 3):
                nc.scalar.dma_start(wq_sb[:, :, j * P:(j + 1) * P], wqS[:, j])
            nc.scalar.dma_start(wo_sb[:], woS[:, :, :])
            for nt in (2, 3):
                for h in range(2):
                    nc.scalar.dma_start(x_sb[:, h * 8:(h + 1) * 8,
                                             nt * 512:(nt + 1) * 512],
                                        xS[:, nt, h * 8:(h + 1) * 8, :])

            def rope(dst_ap, ps, nt):
                """cast psum->bf16, rotate halves, combine with cos/sin tables"""
                raw = work.tile([P, 512], BF16, tag="ropraw")
                nc.scalar.copy(raw[:], ps[:])
                rot = work.tile([P, 512], BF16, tag="roprot")
                for h in range(2):
                    b0 = h * 64
                    nc.sync.dma_start(rot[b0:b0 + 32, :], raw[b0 + 32:b0 + 64, :])
                    nc.sync.dma_start(rot[b0 + 32:b0 + 64, :], raw[b0:b0 + 32, :])
                ts = slice(nt * 512, (nt + 1) * 512)
                t1 = work.tile([P, 512], BF16, tag="ropt1")
                nc.vector.tensor_mul(t1[:], raw[:], cos_sb[:, ts])
                nc.vector.tensor_mul(rot[:], rot[:], sin_sb[:, ts])
                nc.vector.tensor_add(dst_ap, t1[:], rot[:])

            def k_proj(nt):
                ps = big_ps.tile([P, 512], F32, tag="big")
                for ko in range(KO):
                    nc.tensor.matmul(ps[:], wk_sb[:, ko, :],
                                     x_sb[:, ko, nt * 512:(nt + 1) * 512],
                                     start=(ko == 0), stop=(ko == KO - 1))
                rope(kt[:, nt * 512:(nt + 1) * 512], ps, nt)

            def v_proj(nt):
                # feature-major projection (N=512 moving) then DMA-engine
                # transposes flip each 128-token block to token-major v_sb
                ps = big_ps.tile([P, 512], F32, tag="big")
                for ko in range(KO):
                    nc.tensor.matmul(ps[:], wv_sb[:, ko, :],
                                     x_sb[:, ko, nt * 512:(nt + 1) * 512],
                                     start=(ko == 0), stop=(ko == KO - 1))
                vt = work.tile([P, 512], BF16, tag="vt")
                nc.vector.tensor_copy(vt[:], ps[:])
                for t4 in range(4):
                    tt = 4 * nt + t4
                    # XBAR-transpose needs 128B-aligned dst: go via an aligned
                    # scratch, then one strided copy into the 65-stride v_sb
                    vtt = work.tile([P, 128], BF16, tag="vtt")
                    nc.sync.dma_start_transpose(
                        vtt[:, 0:64], vt[0:64, t4 * P:(t4 + 1) * P])
                    nc.sync.dma_start_transpose(
                        vtt[:, 64:128], vt[64:128, t4 * P:(t4 + 1) * P])
                    nc.vector.tensor_copy(
                        v_sb[:, tt, :].rearrange("p (two f) -> p two f",
                                                 two=2)[:, :, 0:64],
                        vtt[:].rearrange("p (two f) -> p two f", two=2))

            def q_proj(j, nt):
                ps = big_ps.tile([P, 512], F32, tag="big")
                for ko in range(KO):
                    nc.tensor.matmul(ps[:], wq_sb[:, ko, j * P:(j + 1) * P],
                                     x_sb[:, ko, nt * 512:(nt + 1) * 512],
                                     start=(ko == 0), stop=(ko == KO - 1))
                rope(qts[j][:, nt * 512:(nt + 1) * 512], ps, nt)

            # ---- Wo micro-op queue: one 512-col matmul (or finalize) per op,
            # popped into the PE gaps of the ACT-paced attention stream ----
            wo_q = deque()
            reserve = [0]

            def pop_wo(n):
                for _ in range(n):
                    if len(wo_q) > reserve[0]:
                        wo_q.popleft()()

            def drain_wo():
                reserve[0] = 0
                while wo_q:
                    wo_q.popleft()()

            def make_wo_ops(qt, ot):
                ops = []
                for tt in range(4):
                    for oc in range(4):
                        box = {}

                        def op_start(box=box, tt=tt, oc=oc, ot=ot):
                            box["ps"] = big_ps.tile([P, 512], F32, tag="big",
                                                    name="wops")
                            nc.tensor.matmul(
                                box["ps"][:], ot[:, 0, tt * P:(tt + 1) * P],
                                wo_sb[:, 0, oc * 512:(oc + 1) * 512],
                                start=True, stop=False)
                        ops.append(op_start)
                        for kf in range(1, 4):
                            def op_mid(box=box, kf=kf, tt=tt, oc=oc, ot=ot):
                                nc.tensor.matmul(
                                    box["ps"][:], ot[:, kf, tt * P:(tt + 1) * P],
                                    wo_sb[:, kf, oc * 512:(oc + 1) * 512],
                                    start=False, stop=(kf == 3))
                            ops.append(op_mid)

                        def op_fin(box=box, qt=qt, tt=tt, oc=oc):
                            ysb = work.tile([P, 512], BF16, tag="ysb")
                            nc.vector.tensor_copy(ysb[:], box["ps"][:])
                            r0 = qt * 512 + tt * P
                            nc.sync.dma_start(
                                y[r0:r0 + P, oc * 512:(oc + 1) * 512], ysb[:])
                        ops.append(op_fin)
                return ops

            # ---- attention for one (qt, j) head-pair into ot tile ----
            def attn_block(qt, j, ot):
                pv = pv_ps.tile([65, 1024], F32, tag="pv")
                pvv = pv[:].rearrange("p (two t) -> p two t", two=2)
                nkb = 4 * qt + 4

                def flush_pv(prev):
                    # PV matmuls for the previous kb (software pipeline: issued
                    # after the next kb's scores so PE never waits on ACT's exp
                    # of the current block). Diagonal blocks only touch output
                    # columns >= their first causally-valid query.
                    pkb, c0, pp = prev
                    ppv = pp[:].rearrange("p (two t) -> p two t", two=2)
                    nc.tensor.matmul(pv[:, c0:512], v_sb[:, pkb, 0:65],
                                     ppv[:, 0, c0:512],
                                     start=(pkb == 0), stop=(pkb == nkb - 1))
                    nc.tensor.matmul(pv[:, 512 + c0:1024], v_sb[:, pkb, 65:130],
                                     ppv[:, 1, c0:512],
                                     start=(pkb == 0), stop=(pkb == nkb - 1))

                pending = []
                for kb in range(nkb):
                    tk = slice(kb * P, (kb + 1) * P)
                    jr = kb - 4 * qt           # >= 0 on diagonal blocks
                    c0 = max(0, jr) * P        # first causally-valid column
                    tqs = slice(qt * 512 + c0, (qt + 1) * 512)
                    # one 2-bank psum tile holds both heads' scores; the two
                    # matmuls land on disjoint PE row halves and run
                    # concurrently, then a SINGLE exp (3-dim AP) and a single
                    # broadcast mask cover both halves
                    sp = s_ps.tile([P, 1024], F32, tag="s")
                    spv = sp[:].rearrange("p (two t) -> p two t", two=2)
                    nc.tensor.matmul(sp[:, c0:512], kt[0:64, tk],
                                     qts[j][0:64, tqs], start=True, stop=True)
                    nc.tensor.matmul(sp[:, 512 + c0:1024], kt[64:128, tk],
                                     qts[j][64:128, tqs], start=True, stop=True)
                    if len(pending) >= 2:
                        flush_pv(pending.pop(0))
                    pp = pexp.tile([P, 1024], BF16, tag="p")
                    ppv = pp[:].rearrange("p (two t) -> p two t", two=2)
                    nc.scalar.activation(ppv[:, :, c0:512], spv[:, :, c0:512],
                                         AF.Exp, scale=SCALE)
                    if jr >= 0:
                        # triangle mask on the one partially-valid block
                        nc.vector.tensor_mul(
                            ppv[:, :, c0:c0 + P], ppv[:, :, c0:c0 + P],
                            tri_sb[:, None, :].to_broadcast((P, 2, P)))
                    pending.append((kb, c0, pp))
                    if c0 == 0:
                        # full-width exp block: ~460ns of PE slack -> two Wo ops
                        pop_wo(2)
                for pr in pending:
                    flush_pv(pr)
                # fast pv release: stage both heads' numerators into ot and
                # the sumexp row out of PSUM, then normalize via a broadcast
                # of the reciprocal denominators
                nc.vector.tensor_copy(ot[0:64, j, :], pvv[0:64, 0, :])
                nc.vector.tensor_copy(ot[64:128, j, :], pvv[0:64, 1, :])
                if qt == 3 and j == 3:
                    # tail block: PE-matmul broadcast (low latency, PE is idle
                    # here) instead of the DRAM bounce
                    srb = rrp.tile([1, 1024], BF16, tag="srb")
                    nc.vector.tensor_copy(srb[:], pv[64:65, :])
                    bc_ps = big_ps.tile([P, 512], F32, tag="big")
                    nc.tensor.matmul(bc_ps[0:64, :], ones_sb[0:1, :],
                                     srb[0:1, 0:512], start=True, stop=True)
                    nc.tensor.matmul(bc_ps[64:128, :], ones_sb[0:1, :],
                                     srb[0:1, 512:1024], start=True, stop=True)
                    rec = rrp.tile([P, 512], F32, tag="bc", name="recf")
                    nc.vector.reciprocal_approx_fast(rec[:], bc_ps[:])
                else:
                    srow = rrp.tile([1, 1024], F32, tag="sr")
                    nc.vector.tensor_copy(srow[:], pv[64:65, :])
                    drx = dramp.tile([1, 1024], F32, tag="drx")
                    nc.sync.dma_start(drx[:], srow[:])
                    rec = rrp.tile([P, 512], F32, tag="bc", name="bc")
                    for idx in range(2):
                        nc.sync.dma_start(
                            rec[idx * 64:(idx + 1) * 64, :],
                            drx[0, idx * 512:(idx + 1) * 512][None, :]
                            .to_broadcast((64, 512)))
                    nc.vector.reciprocal_approx_fast(rec[:], rec[:])
                for idx in range(2):
                    nc.vector.tensor_mul(ot[idx * 64:(idx + 1) * 64, j, :],
                                         ot[idx * 64:(idx + 1) * 64, j, :],
                                         rec[idx * 64:(idx + 1) * 64, :])

            # ---- phase loop: stream over 512-token windows ----
            ot_tiles = {}
            for nt in range(NT):
                # hold back a few Wo ops on the last phase so the PE has work
                # to chew while the final normalization chain resolves
                reserve[0] = 20 if nt == 3 else 0
                ot_tiles[nt] = otp.tile([P, 4, 512], BF16, tag="ot",
                                        name=f"ot{nt}")
                if nt == 0:
                    k_proj(0)
                    v_proj(0)
                q_proj(0, nt)
                attn_block(nt, 0, ot_tiles[nt])
                if nt < 3:
                    k_proj(nt + 1)
                    v_proj(nt + 1)
                for j in range(1, 4):
                    q_proj(j, nt)
                    attn_block(nt, j, ot_tiles[nt])
                drain_wo()
                wo_q.extend(make_wo_ops(nt, ot_tiles[nt]))
            drain_wo()

    nc.finalize()
    _nc_cache["nc"] = nc
    return nc


def make_in_maps(x, Wq, Wk, Wv, Wo):
    bf = ml_dtypes.bfloat16
    x = np.asarray(x, np.float32)
    Wq = np.asarray(Wq, np.float32)
    Wk = np.asarray(Wk, np.float32)
    Wv = np.asarray(Wv, np.float32)
    Wo = np.asarray(Wo, np.float32)

    # rope tables, [128, T]: row p covers head-dim d = p % 64
    half = HD // 2
    inv_freq = 1.0 / (ROPE_BASE ** (np.arange(half, dtype=np.float64) / half))
    pos = np.arange(T, dtype=np.float64)
    d_idx = np.arange(P) % HD
    freqs = pos[None, :] * inv_freq[d_idx % half][:, None]      # [128, T]
    cos_t = np.cos(freqs).astype(bf)
    sign = np.where(d_idx < half, -1.0, 1.0)[:, None]
    sin_t = (np.sin(freqs) * sign).astype(bf)

    # causal 0/1 triangle for the partially-valid diagonal sub-block
    pp = np.arange(P)[:, None]
    ff = np.arange(P)[None, :]
    tri = (ff >= pp).astype(bf)

    in_maps = []
    for c in range(8):
        b, g = c // 4, c % 4
        heads = [8 * g + h for h in PERM_Q]
        qrows = np.concatenate([np.arange(h * HD, (h + 1) * HD) for h in heads])
        kvrows = np.arange(2 * g * HD, (2 * g + 2) * HD)
        xT = x[b].T                                              # [D, T]
        xS = xT.reshape(KO, P, NT, 512).transpose(1, 2, 0, 3)    # [P,NT,KO,512]
        wqS = (Wq[qrows, :].T.reshape(KO, P, 4, P)
               .transpose(1, 2, 0, 3))                           # [P,4,KO,128]
        wkS = Wk[kvrows, :].T.reshape(KO, P, DKV).transpose(1, 0, 2)
        wvS = Wv[kvrows, :].T.reshape(KO, P, DKV).transpose(1, 0, 2)
        woS = Wo[:, qrows].T.reshape(4, P, D).transpose(1, 0, 2)  # [P,4,D]
        in_maps.append({
            "xS": np.ascontiguousarray(xS).astype(bf),
            "wqS": np.ascontiguousarray(wqS).astype(bf),
            "wkS": np.ascontiguousarray(wkS).astype(bf),
            "wvS": np.ascontiguousarray(wvS).astype(bf),
            "woS": np.ascontiguousarray(woS).astype(bf),
            "cosT": cos_t,
            "sinT": sin_t,
            "tri": tri,
        })
    return in_maps


def combine_outputs(results):
    out = np.zeros((B, T, D), np.float32)
    for c in range(8):
        out[c // 4] += results[c]["y"].astype(np.float32)
    return out


def _ensure_ntff_hook():
    """Register the axon NTFF profile hook (antenv.axon_hooks is missing
    from this image; recreate it and wire the ctypes hook from trn_boot)."""
    import sys, types
    if "antenv.axon_hooks" in sys.modules:
        return
    m = types.ModuleType("antenv.axon_hooks")
    hook = [None]
    m.set_axon_ntff_profile_hook = lambda h: hook.__setitem__(0, h)
    m.get_axon_ntff_profile_hook = lambda: hook[0]
    sys.modules["antenv.axon_hooks"] = m
    import antenv
    antenv.axon_hooks = m
    sys.path.insert(0, "/root/.axon_site")
    from trn_agent_boot.trn_boot import _ntff_profile_via_ctypes
    m.set_axon_ntff_profile_hook(
        _ntff_profile_via_ctypes("/opt/axon/libaxon_pjrt.so"))


def kernel(x, Wq, Wk, Wv, Wo, _trace=False):
    if _trace:
        _ensure_ntff_hook()
    nc = build_nc()
    in_maps = make_in_maps(x, Wq, Wk, Wv, Wo)
    res = run_bass_kernel_spmd(nc, in_maps, core_ids=list(range(8)), trace=_trace)
    out = combine_outputs(res.results)
    if _trace:
        return out, res
    return out


# revision 16
# speedup vs baseline: 1.0043x; 1.0043x over previous
"""Distributed GQA attention kernel for Trainium2 (8 NeuronCores).

Sharding: 2-way data parallel over batch x 4-way tensor parallel over heads.
Core c handles batch b = c // 4 and head group g = c % 4 (8 q-heads, 2 kv-heads).
Each core computes a full-size partial of the output (its head group pushed
through Wo); the host sums the 4 partials per batch. No on-device collective.

Device-side layout is feature-major (Q^T/K^T: [feature partitions, T free]) so
projections consume the host-pre-transposed x^T directly, attention scores are
computed transposed (S^T[tk, tq]) so softmax(P)@V needs no transposes, and the
softmax denominator comes free from an appended ones-column on V.

Schedule: x streams in 512-token column windows; each phase nt runs
K/V-projection prefetch for window nt+1, Q-projection + attention for q-tile
nt, and interleaves the previous tile's Wo matmuls into the per-block PE gaps
of the ACT(exp)-paced attention stream. V is projected feature-major (big
moving dim) and flipped token-major by DMA-engine transposes.
"""

import numpy as np
import ml_dtypes
from collections import deque
from contextlib import ExitStack

import concourse.bass as bass
from concourse import bacc
import concourse.mybir as mybir
import concourse.tile as tile
from concourse.bass_utils import run_bass_kernel_spmd

BF16 = mybir.dt.bfloat16
F32 = mybir.dt.float32
AF = mybir.ActivationFunctionType

P = 128
B, T, D = 2, 2048, 2048
NUM_HEADS, NUM_KV_HEADS, HD = 32, 8, 64
FQ = 512          # q features per core (8 heads x 64)
DKV = 128         # kv features per core (2 kv heads x 64)
KO = D // P       # 16 contraction tiles over d_model
NT = T // 512     # 4 tiles of 512 along T
SCALE = 1.0 / np.sqrt(HD)
ROPE_BASE = 10000.0
# local head order inside the 512 q-features: pairs (j, j+4) so that the two
# heads in partition tile j sit at bases 0/64 matching kv heads 0/1 in K^T
PERM_Q = [0, 4, 1, 5, 2, 6, 3, 7]

_nc_cache = {}


def build_nc():
    if "nc" in _nc_cache:
        return _nc_cache["nc"]
    nc = bacc.Bacc()
    xS = nc.declare_dram_parameter("xS", [P, NT, KO, 512], BF16, isOutput=False)
    wqS = nc.declare_dram_parameter("wqS", [P, 4, KO, P], BF16, isOutput=False)
    wkS = nc.declare_dram_parameter("wkS", [P, KO, DKV], BF16, isOutput=False)
    wvS = nc.declare_dram_parameter("wvS", [P, KO, DKV], BF16, isOutput=False)
    woS = nc.declare_dram_parameter("woS", [P, 4, D], BF16, isOutput=False)
    cosd = nc.declare_dram_parameter("cosT", [P, T], BF16, isOutput=False)
    sind = nc.declare_dram_parameter("sinT", [P, T], BF16, isOutput=False)
    mskd = nc.declare_dram_parameter("tri", [P, P], BF16, isOutput=False)
    y = nc.declare_dram_parameter("y", [T, D], BF16, isOutput=True)

    with tile.TileContext(nc) as tc:
        with ExitStack() as ctx:
            const = ctx.enter_context(tc.tile_pool(name="const", bufs=1))
            work = ctx.enter_context(tc.tile_pool(name="work", bufs=4))
            otp = ctx.enter_context(tc.tile_pool(name="otp", bufs=2))
            pexp = ctx.enter_context(tc.tile_pool(name="pexp", bufs=8))
            rrp = ctx.enter_context(tc.tile_pool(name="rrp", bufs=2))
            dramp = ctx.enter_context(tc.tile_pool(name="dramp", bufs=2, space="DRAM"))
            big_ps = ctx.enter_context(tc.tile_pool(name="bigps", bufs=2, space="PSUM"))
            pv_ps = ctx.enter_context(tc.tile_pool(name="pvps", bufs=1, space="PSUM"))
            s_ps = ctx.enter_context(tc.tile_pool(name="sps", bufs=2, space="PSUM"))

            # ---- persistent tiles ----
            wk_sb = const.tile([P, KO, DKV], BF16, tag="wk")
            wv_sb = const.tile([P, KO, DKV], BF16, tag="wv")
            wq_sb = const.tile([P, KO, FQ], BF16, tag="wq")
            wo_sb = const.tile([P, 4, D], BF16, tag="wo")
            cos_sb = const.tile([P, T], BF16, tag="cos")
            sin_sb = const.tile([P, T], BF16, tag="sin")
            tri_sb = const.tile([P, P], BF16, tag="tri")
            # window-major x so each 512-token window is one contiguous
            # 16KB-per-partition DMA (single descriptor line -> cheap issue)
            x_sb = const.tile([P, NT, KO, 512], BF16, tag="x")
            kt = const.tile([P, T], BF16, tag="kt")
            v_sb = const.tile([P, 16, 130], BF16, tag="v")
            ones_sb = const.tile([1, 64], BF16, tag="ones")
            qts = {j: const.tile([P, T], BF16, tag=f"qt{j}", name=f"qt{j}")
                   for j in range(4)}

            nc.gpsimd.memset(ones_sb[:], 1.0)
            nc.gpsimd.memset(v_sb[:, :, 64:65], 1.0)
            nc.gpsimd.memset(v_sb[:, :, 129:130], 1.0)

            # ---- startup bulk loads ----
            # Bulk goes on the Scalar HWDGE ring (so it never queues ahead of
            # the latency-critical small DMAs on the Sync ring); only what
            # phase 0 needs first is issued here, the rest is issued at
            # ACT-slack points inside the phase loop below.
            nc.scalar.dma_start(wk_sb[:], wkS[:, :, :])
            nc.scalar.dma_start(x_sb[:, 0, 0:8, :], xS[:, 0, 0:8, :])
            nc.scalar.dma_start(x_sb[:, 0, 8:16, :], xS[:, 0, 8:16, :])
            nc.scalar.dma_start(cos_sb[:], cosd[:])
            nc.scalar.dma_start(sin_sb[:], sind[:])

            def rope(dst_ap, ps, nt):
                """cast psum->bf16, rotate halves, combine with cos/sin tables"""
                raw = work.tile([P, 512], BF16, tag="ropraw")
                nc.scalar.copy(raw[:], ps[:])
                rot = work.tile([P, 512], BF16, tag="roprot")
                for h in range(2):
                    b0 = h * 64
                    nc.sync.dma_start(rot[b0:b0 + 32, :], raw[b0 + 32:b0 + 64, :])
                    nc.sync.dma_start(rot[b0 + 32:b0 + 64, :], raw[b0:b0 + 32, :])
                ts = slice(nt * 512, (nt + 1) * 512)
                t1 = work.tile([P, 512], BF16, tag="ropt1")
                nc.vector.tensor_mul(t1[:], raw[:], cos_sb[:, ts])
                nc.vector.tensor_mul(rot[:], rot[:], sin_sb[:, ts])
                nc.vector.tensor_add(dst_ap, t1[:], rot[:])

            def k_proj(nt):
                ps = big_ps.tile([P, 512], F32, tag="big")
                for ko in range(KO):
                    nc.tensor.matmul(ps[:], wk_sb[:, ko, :],
                                     x_sb[:, nt, ko, :],
                                     start=(ko == 0), stop=(ko == KO - 1))
                rope(kt[:, nt * 512:(nt + 1) * 512], ps, nt)

            def v_proj(nt):
                # feature-major projection (N=512 moving) then DMA-engine
                # transposes flip each 128-token block to token-major v_sb
                ps = big_ps.tile([P, 512], F32, tag="big")
                for ko in range(KO):
                    nc.tensor.matmul(ps[:], wv_sb[:, ko, :],
                                     x_sb[:, nt, ko, :],
                                     start=(ko == 0), stop=(ko == KO - 1))
                vt = work.tile([P, 512], BF16, tag="vt")
                nc.vector.tensor_copy(vt[:], ps[:])
                for t4 in range(4):
                    tt = 4 * nt + t4
                    # XBAR-transpose needs 128B-aligned dst: go via an aligned
                    # scratch, then one strided copy into the 65-stride v_sb
                    vtt = work.tile([P, 128], BF16, tag="vtt")
                    nc.sync.dma_start_transpose(
                        vtt[:, 0:64], vt[0:64, t4 * P:(t4 + 1) * P])
                    nc.sync.dma_start_transpose(
                        vtt[:, 64:128], vt[64:128, t4 * P:(t4 + 1) * P])
                    nc.vector.tensor_copy(
                        v_sb[:, tt, :].rearrange("p (two f) -> p two f",
                                                 two=2)[:, :, 0:64],
                        vtt[:].rearrange("p (two f) -> p two f", two=2))

            def q_proj(j, nt):
                ps = big_ps.tile([P, 512], F32, tag="big")
                for ko in range(KO):
                    nc.tensor.matmul(ps[:], wq_sb[:, ko, j * P:(j + 1) * P],
                                     x_sb[:, nt, ko, :],
                                     start=(ko == 0), stop=(ko == KO - 1))
                rope(qts[j][:, nt * 512:(nt + 1) * 512], ps, nt)

            # ---- Wo micro-op queue: one 512-col matmul (or finalize) per op,
            # popped into the PE gaps of the ACT-paced attention stream ----
            wo_q = deque()
            reserve = [0]

            def pop_wo(n):
                for _ in range(n):
                    if len(wo_q) > reserve[0]:
                        wo_q.popleft()()

            def drain_wo():
                reserve[0] = 0
                while wo_q:
                    wo_q.popleft()()

            def make_wo_ops(qt, ot):
                ops = []
                for tt in range(4):
                    for oc in range(4):
                        box = {}

                        def op_start(box=box, tt=tt, oc=oc, ot=ot):
                            box["ps"] = big_ps.tile([P, 512], F32, tag="big",
                                                    name="wops")
                            nc.tensor.matmul(
                                box["ps"][:], ot[:, 0, tt * P:(tt + 1) * P],
                                wo_sb[:, 0, oc * 512:(oc + 1) * 512],
                                start=True, stop=False)
                        ops.append(op_start)
                        for kf in range(1, 4):
                            def op_mid(box=box, kf=kf, tt=tt, oc=oc, ot=ot):
                                nc.tensor.matmul(
                                    box["ps"][:], ot[:, kf, tt * P:(tt + 1) * P],
                                    wo_sb[:, kf, oc * 512:(oc + 1) * 512],
                                    start=False, stop=(kf == 3))
                            ops.append(op_mid)

                        def op_fin(box=box, qt=qt, tt=tt, oc=oc):
                            ysb = work.tile([P, 512], BF16, tag="ysb")
                            nc.vector.tensor_copy(ysb[:], box["ps"][:])
                            r0 = qt * 512 + tt * P
                            nc.sync.dma_start(
                                y[r0:r0 + P, oc * 512:(oc + 1) * 512], ysb[:])
                        ops.append(op_fin)
                return ops

            # ---- attention for one (qt, j) head-pair into ot tile ----
            def attn_block(qt, j, ot):
                pv = pv_ps.tile([65, 1024], F32, tag="pv")
                pvv = pv[:].rearrange("p (two t) -> p two t", two=2)
                nkb = 4 * qt + 4

                def flush_pv(prev):
                    # PV matmuls for the previous kb (software pipeline: issued
                    # after the next kb's scores so PE never waits on ACT's exp
                    # of the current block). Diagonal blocks only touch output
                    # columns >= their first causally-valid query.
                    pkb, c0, pp = prev
                    ppv = pp[:].rearrange("p (two t) -> p two t", two=2)
                    nc.tensor.matmul(pv[:, c0:512], v_sb[:, pkb, 0:65],
                                     ppv[:, 0, c0:512],
                                     start=(pkb == 0), stop=(pkb == nkb - 1))
                    nc.tensor.matmul(pv[:, 512 + c0:1024], v_sb[:, pkb, 65:130],
                                     ppv[:, 1, c0:512],
                                     start=(pkb == 0), stop=(pkb == nkb - 1))

                pending = []
                for kb in range(nkb):
                    tk = slice(kb * P, (kb + 1) * P)
                    jr = kb - 4 * qt           # >= 0 on diagonal blocks
                    c0 = max(0, jr) * P        # first causally-valid column
                    tqs = slice(qt * 512 + c0, (qt + 1) * 512)
                    # one 2-bank psum tile holds both heads' scores; the two
                    # matmuls land on disjoint PE row halves and run
                    # concurrently, then a SINGLE exp (3-dim AP) and a single
                    # broadcast mask cover both halves
                    sp = s_ps.tile([P, 1024], F32, tag="s")
                    spv = sp[:].rearrange("p (two t) -> p two t", two=2)
                    nc.tensor.matmul(sp[:, c0:512], kt[0:64, tk],
                                     qts[j][0:64, tqs], start=True, stop=True)
                    nc.tensor.matmul(sp[:, 512 + c0:1024], kt[64:128, tk],
                                     qts[j][64:128, tqs], start=True, stop=True)
                    if len(pending) >= 2:
                        flush_pv(pending.pop(0))
                    pp = pexp.tile([P, 1024], BF16, tag="p")
                    ppv = pp[:].rearrange("p (two t) -> p two t", two=2)
                    nc.scalar.activation(ppv[:, :, c0:512], spv[:, :, c0:512],
                                         AF.Exp, scale=SCALE)
                    if jr >= 0:
                        # triangle mask on the one partially-valid block
                        nc.vector.tensor_mul(
                            ppv[:, :, c0:c0 + P], ppv[:, :, c0:c0 + P],
                            tri_sb[:, None, :].to_broadcast((P, 2, P)))
                    pending.append((kb, c0, pp))
                    if c0 == 0:
                        # full-width exp block: ~460ns of PE slack -> two Wo ops
                        pop_wo(2)
                for pr in pending:
                    flush_pv(pr)
                # fast pv release: stage both heads' numerators into ot and
                # the sumexp row out of PSUM, then normalize via a broadcast
                # of the reciprocal denominators
                nc.vector.tensor_copy(ot[0:64, j, :], pvv[0:64, 0, :])
                nc.vector.tensor_copy(ot[64:128, j, :], pvv[0:64, 1, :])
                if qt == 3 and j == 3:
                    # tail block: PE-matmul broadcast (low latency, PE is idle
                    # here) instead of the DRAM bounce
                    srb = rrp.tile([1, 1024], BF16, tag="srb")
                    nc.vector.tensor_copy(srb[:], pv[64:65, :])
                    bc_ps = big_ps.tile([P, 512], F32, tag="big")
                    nc.tensor.matmul(bc_ps[0:64, :], ones_sb[0:1, :],
                                     srb[0:1, 0:512], start=True, stop=True)
                    nc.tensor.matmul(bc_ps[64:128, :], ones_sb[0:1, :],
                                     srb[0:1, 512:1024], start=True, stop=True)
                    rec = rrp.tile([P, 512], F32, tag="bc", name="recf")
                    nc.vector.reciprocal_approx_fast(rec[:], bc_ps[:])
                else:
                    srow = rrp.tile([1, 1024], F32, tag="sr")
                    nc.vector.tensor_copy(srow[:], pv[64:65, :])
                    drx = dramp.tile([1, 1024], F32, tag="drx")
                    nc.sync.dma_start(drx[:], srow[:])
                    rec = rrp.tile([P, 512], F32, tag="bc", name="bc")
                    for idx in range(2):
                        nc.sync.dma_start(
                            rec[idx * 64:(idx + 1) * 64, :],
                            drx[0, idx * 512:(idx + 1) * 512][None, :]
                            .to_broadcast((64, 512)))
                    nc.vector.reciprocal_approx_fast(rec[:], rec[:])
                for idx in range(2):
                    nc.vector.tensor_mul(ot[idx * 64:(idx + 1) * 64, j, :],
                                         ot[idx * 64:(idx + 1) * 64, j, :],
                                         rec[idx * 64:(idx + 1) * 64, :])

            # ---- phase loop: stream over 512-token windows ----
            ot_tiles = {}
            for nt in range(NT):
                # hold back a few Wo ops on the last phase so the PE has work
                # to chew while the final normalization chain resolves
                reserve[0] = 20 if nt == 3 else 0
                ot_tiles[nt] = otp.tile([P, 4, 512], BF16, tag="ot",
                                        name=f"ot{nt}")
                if nt == 0:
                    k_proj(0)
                    # remaining bulk streams in at ACT-slack points: each
                    # issue costs the Scalar engine ~1-3us of descriptor
                    # generation, so spread them between exp bursts
                    nc.scalar.dma_start(wv_sb[:], wvS[:, :, :])
                    nc.scalar.dma_start(wq_sb[:, :, 0:P], wqS[:, 0])
                    nc.scalar.dma_start(tri_sb[:], mskd[:])
                    v_proj(0)
                q_proj(0, nt)
                if nt == 0:
                    nc.scalar.dma_start(x_sb[:, 1, :, :], xS[:, 1, :, :])
                attn_block(nt, 0, ot_tiles[nt])
                if nt == 0:
                    for j2 in (1, 2, 3):
                        nc.scalar.dma_start(wq_sb[:, :, j2 * P:(j2 + 1) * P],
                                            wqS[:, j2])
                if nt == 1:
                    nc.scalar.dma_start(x_sb[:, 3, :, :], xS[:, 3, :, :])
                if nt < 3:
                    k_proj(nt + 1)
                    v_proj(nt + 1)
                for j in range(1, 4):
                    q_proj(j, nt)
                    attn_block(nt, j, ot_tiles[nt])
                    if nt == 0 and j == 1:
                        nc.scalar.dma_start(wo_sb[:], woS[:, :, :])
                    if nt == 0 and j == 2:
                        nc.scalar.dma_start(x_sb[:, 2, :, :], xS[:, 2, :, :])
                drain_wo()
                wo_q.extend(make_wo_ops(nt, ot_tiles[nt]))
            drain_wo()

    nc.finalize()
    _nc_cache["nc"] = nc
    return nc


def make_in_maps(x, Wq, Wk, Wv, Wo):
    bf = ml_dtypes.bfloat16
    x = np.asarray(x, np.float32)
    Wq = np.asarray(Wq, np.float32)
    Wk = np.asarray(Wk, np.float32)
    Wv = np.asarray(Wv, np.float32)
    Wo = np.asarray(Wo, np.float32)

    # rope tables, [128, T]: row p covers head-dim d = p % 64
    half = HD // 2
    inv_freq = 1.0 / (ROPE_BASE ** (np.arange(half, dtype=np.float64) / half))
    pos = np.arange(T, dtype=np.float64)
    d_idx = np.arange(P) % HD
    freqs = pos[None, :] * inv_freq[d_idx % half][:, None]      # [128, T]
    cos_t = np.cos(freqs).astype(bf)
    sign = np.where(d_idx < half, -1.0, 1.0)[:, None]
    sin_t = (np.sin(freqs) * sign).astype(bf)

    # causal 0/1 triangle for the partially-valid diagonal sub-block
    pp = np.arange(P)[:, None]
    ff = np.arange(P)[None, :]
    tri = (ff >= pp).astype(bf)

    in_maps = []
    for c in range(8):
        b, g = c // 4, c % 4
        heads = [8 * g + h for h in PERM_Q]
        qrows = np.concatenate([np.arange(h * HD, (h + 1) * HD) for h in heads])
        kvrows = np.arange(2 * g * HD, (2 * g + 2) * HD)
        xT = x[b].T                                              # [D, T]
        xS = xT.reshape(KO, P, NT, 512).transpose(1, 2, 0, 3)    # [P,NT,KO,512]
        wqS = (Wq[qrows, :].T.reshape(KO, P, 4, P)
               .transpose(1, 2, 0, 3))                           # [P,4,KO,128]
        wkS = Wk[kvrows, :].T.reshape(KO, P, DKV).transpose(1, 0, 2)
        wvS = Wv[kvrows, :].T.reshape(KO, P, DKV).transpose(1, 0, 2)
        woS = Wo[:, qrows].T.reshape(4, P, D).transpose(1, 0, 2)  # [P,4,D]
        in_maps.append({
            "xS": np.ascontiguousarray(xS).astype(bf),
            "wqS": np.ascontiguousarray(wqS).astype(bf),
            "wkS": np.ascontiguousarray(wkS).astype(bf),
            "wvS": np.ascontiguousarray(wvS).astype(bf),
            "woS": np.ascontiguousarray(woS).astype(bf),
            "cosT": cos_t,
            "sinT": sin_t,
            "tri": tri,
        })
    return in_maps


def combine_outputs(results):
    out = np.zeros((B, T, D), np.float32)
    for c in range(8):
        out[c // 4] += results[c]["y"].astype(np.float32)
    return out


def _ensure_ntff_hook():
    """Register the axon NTFF profile hook (antenv.axon_hooks is missing
    from this image; recreate it and wire the ctypes hook from trn_boot)."""
    import sys, types
    if "antenv.axon_hooks" in sys.modules:
        return
    m = types.ModuleType("antenv.axon_hooks")
    hook = [None]
    m.set_axon_ntff_profile_hook = lambda h: hook.__setitem__(0, h)
    m.get_axon_ntff_profile_hook = lambda: hook[0]
    sys.modules["antenv.axon_hooks"] = m
    import antenv
    antenv.axon_hooks = m
    sys.path.insert(0, "/root/.axon_site")
    from trn_agent_boot.trn_boot import _ntff_profile_via_ctypes
    m.set_axon_ntff_profile_hook(
        _ntff_profile_via_ctypes("/opt/axon/libaxon_pjrt.so"))


def kernel(x, Wq, Wk, Wv, Wo, _trace=False):
    if _trace:
        _ensure_ntff_hook()
    nc = build_nc()
    in_maps = make_in_maps(x, Wq, Wk, Wv, Wo)
    res = run_bass_kernel_spmd(nc, in_maps, core_ids=list(range(8)), trace=_trace)
    out = combine_outputs(res.results)
    if _trace:
        return out, res
    return out
